# revision 32
# baseline (speedup 1.0000x reference)
"""ControlCPC loss kernel for 8 Trainium2 NeuronCores (Bass/Tile) — v2.

Row-sharded over the batch: core c owns rows [128c, 128c+128).

Algebraic reduction (B=1024, Z=512, A=8, AH=64, ZH=512, n_neg=100, T=0.1):
  sim[b, j] = z_next[b] . z_next_hat[j]
  u[b]      = z[b] @ W1[:Z] + b1
  G[j]      = relu(actions[j] @ Wa + ba) @ W1[Z:]      (j = s..s+255 local)
  v[b]      = z_next[b] @ W2.T
  neg[b, i] = z[b].z_next[b] + b2.z_next[b]
              + sum_zh relu(u[b,zh] + G[b+i,zh]) * v[b,zh]     i = 1..99
  row stats over x = [sim | neg]: m = rowmax, S = sum exp((x-m)/T),
  diag = sim[b,b] (exact from PSUM), cnt = #{x > diag}.
Host: loss = mean(m/T + ln S - diag/T), acc_k = mean(cnt < k).

v2 structure (vs the 79.6us baseline, cost-model-guided):
- t_i = u + G-window built by TWO fp8 DoubleRow matmuls per shift (~214ns
  hot vs 654 for 3 bf16 matmuls): rhs operands are plane-packed [p, 2, 512]
  (G1|G0 and u-halves), lhsT selectors are 3D slices of one diagonal
  master [128, 356] fp8 (master[p, p+128]=1), offset = shift.
- the relu.v reduce is spread over three engine routes in 3-shift macros:
    A: Act relu-copies the 3-bank PSUM region to bf16 SBUF (1465ns);
       DVE TT-mult by v (2x mode, 860) + 3 TS-accums (4x mode, 194).
    Z: Act relu-copy; Pool TT-mult (3143); 3 DVE TS-accums.
    X: 3 DVE fused STTs straight from PSUM (658 each).
  (Pool cannot touch PSUM or run STT/TS - BIR engine checks.)
- sim matmuls + stats and all input DMAs are overlapped into the loop.
fp8 e4m3 on u/G adds ~0.4 rms error per neg value; the loss tolerance
(rel 2e-2 on |loss|~887) and top-k margins absorb it (measured ~1e-3).
"""

import sys

for _p in ("/opt/trn_rl_repo", "/opt/pypackages"):
    if _p not in sys.path:
        sys.path.insert(0, _p)

import numpy as np
import ml_dtypes

import concourse.bass as bass
import concourse.mybir as mybir
import concourse.tile as tile
from concourse import bacc
from concourse.bass_utils import run_bass_kernel_spmd

f32 = mybir.dt.float32
bf16 = mybir.dt.bfloat16
f8e4 = mybir.dt.float8e4
AL = mybir.AluOpType
AF = mybir.ActivationFunctionType

B, Z, A = 1024, 512, 8
AH, ZH = 64, 512
TEMP = 0.1
NCORES = 8
R = B // NCORES          # 128 rows per core
NSH = 99                 # shifts 1..99
SH_PER = 2               # shifts per macro
NMAC = 50                # ceil(99/2); last macro carries 1 shift
INV_T = 1.0 / TEMP
TOP_K = (1, 3, 10)
DW = 1920                # simdump row pitch; diag at flat[b*(DW+1)]
SOFF = 0                 # znhT is host-rotated per core so diag sits at col b

# macro routes: A = Act+DVE chain, Z = Act+Pool chain, X = DVE fused-PSUM.
# Proportional interleave tuned against the TimelineSim cost model.
def _make_pattern(counts):
    err = {r: 0.0 for r in counts}
    total = sum(counts.values())
    out = []
    for _ in range(total):
        for r in counts:
            err[r] += counts[r] / total
        pick = max(err, key=lambda r: err[r])
        err[pick] -= 1.0
        out.append(pick)
    return "".join(out)

ROUTE_PATTERN = ("ZZ" + _make_pattern({"Z": 16, "A": 15, "X": 13}) + "AXAX")
assert len(ROUTE_PATTERN) == NMAC

_cache = {}


def _register_diag_op():
    from concourse.dve_ops import DveOp, OPS
    from concourse.dve_spec import Spec, Src0, C0, Zero, Idx, eq, select
    from operator import add

    for op in OPS:
        if op.name == "DIAG_EXTRACT_ANT":
            return op

    def _ref(in0, in1, s0, s1, imm2):
        in2 = in0.reshape(in0.shape[0], -1).astype(np.float32)
        idx = np.arange(in2.shape[1])[None, :]
        sel = np.where(idx == s0, in2, 0.0).astype(np.float32)
        return sel.reshape(in0.shape), sel.sum(axis=-1, keepdims=True)

    import concourse.dve_ops as dve_ops_mod
    from concourse.dve_spec import lower, _has_src1
    from concourse.dve_uop import DveOpSpec

    name = "DIAG_EXTRACT_ANT"
    spec = Spec(body=select(eq(Idx, C0), Src0, Zero), accum=add,
                accum_init=Zero, reference=_ref)
    row = max(dve_ops_mod._SUB_OPCODE_FOR_NAME.values()) + 1
    assert row < 0x20
    dve_ops_mod._SUB_OPCODE_FOR_NAME[name] = row

    shas = {}
    for ver in ("v3", "v4"):
        try:
            tmp = DveOpSpec(name=name, opcode=row, uops=lower(spec, ver=ver),
                            rd1_en=_has_src1(spec))
            shas[ver] = tmp.sha(ver)
        except Exception:
            pass
    op = DveOp(name, spec, subdim=False, uops_sha=shas)
    OPS.append(op)
    dve_ops_mod.CUSTOM_DVE_SPECS[name] = spec
    return op


def _build():
    diag_op = _register_diag_op()
    nc = bacc.Bacc("TRN2", target_bir_lowering=False, debug=False,
                   num_devices=NCORES)

    def din(name, shape, dtype=f32):
        return nc.dram_tensor(name, shape, dtype, kind="ExternalInput")

    d_small = din("small", [A + 1, AH + 2 * R], bf16)  # Wa_aug | aTA
    d_W1a = din("W1a_t", [AH, ZH + 8], bf16)           # W1a | b1 spread rows
    d_b1 = din("b1row", [1, ZH], bf16)
    d_sel = din("sel", [R, 356 + 256], f8e4)   # master | sel_u
    d_zzn = din("zzn_tiled", [R, 4 * R], bf16)  # z_next^T k-tiles
    d_zzz = din("zzz_tiled", [R, 4 * R], bf16)  # z^T k-tiles
    d_W1z = din("W1z_tiled", [R, 4 * ZH], f8e4)
    d_W2T = din("W2T_tiled", [R, 4 * ZH], f8e4)
    d_znhT = din("znhT_tiled", [R, 4 * B], bf16)
    d_zp = din("zpack", [R, 2 * Z])            # z+b2 | z_next

    d_out = nc.dram_tensor("out4", [R, 4], f32, kind="ExternalOutput")

    with tile.TileContext(nc) as tc:
        with tc.tile_pool(name="w", bufs=1) as wp, \
             tc.tile_pool(name="t3p", bufs=6) as t3p, \
             tc.tile_pool(name="rv3p", bufs=7) as rv3p, \
             tc.tile_pool(name="gdram", bufs=1, space="DRAM") as gdp, \
             tc.tile_pool(name="scr", bufs=6) as scrp, \
             tc.tile_pool(name="sm", bufs=1) as smp, \
             tc.tile_pool(name="pssim", bufs=1, space="PSUM") as ps_sim:

            # ---------------- input loads ----------------
            small_t = wp.tile([A + 1, AH + 2 * R], bf16, tag="small")
            nc.sync.dma_start(small_t[:], d_small[:, :])
            WaA_t = small_t[:, 0:AH]
            aTA_t = small_t[:, AH:AH + 2 * R]
            W1a_tile = wp.tile([AH, ZH + 8], bf16, tag="W1a")
            nc.sync.dma_start(W1a_tile[:], d_W1a[:, :])
            W1a_t = W1a_tile[:, 0:ZH]
            b1_tile = wp.tile([1, ZH], bf16, tag="b1")
            nc.sync.dma_start(b1_tile[:], d_b1[:, :])
            b1_t = b1_tile[0:1, :]
            W1z_t = wp.tile([R, 4, ZH], f8e4, tag="W1z")
            nc.sync.dma_start(W1z_t[:], d_W1z[:, :].rearrange(
                "p (k m) -> p k m", k=4))
            zzz_t = wp.tile([R, 4, R], bf16, tag="zzz")
            nc.sync.dma_start(zzz_t[:], d_zzz[:, :].rearrange(
                "p (k m) -> p k m", k=4))
            sel_t = wp.tile([R, 356 + 256], f8e4, tag="sel")
            nc.sync.dma_start(sel_t[:], d_sel[:, :])
            W2T_t = wp.tile([R, 4, ZH], f8e4, tag="W2T")
            nc.sync.dma_start(W2T_t[:], d_W2T[:, :].rearrange(
                "p (k m) -> p k m", k=4))
            zzn_t = wp.tile([R, 4, R], bf16, tag="zzn")
            nc.sync.dma_start(zzn_t[:], d_zzn[:, :].rearrange(
                "p (k m) -> p k m", k=4))
            znhT_t = wp.tile([R, 4, B], bf16, tag="znhT")
            nc.sync.dma_start(znhT_t[:], d_znhT[:, :].rearrange(
                "p (k m) -> p k m", k=4))
            zp_t = wp.tile([R, 2 * Z], f32, tag="zp")
            nc.sync.dma_start(zp_t[:], d_zp[:, :])

            ones_t = wp.tile([1, R], bf16, tag="ones")
            nc.gpsimd.memset(ones_t[:], 1.0)

            # DRAM scratch for the sim diagonal gather (see emit j==22):
            # rows padded to DW so flat[b*(DW+1)] == dump[b, b].
            simdump = gdp.tile([R + 1, DW], bf16, tag="simdump")

            # ---------------- setup compute ----------------
            with tc.tile_pool(name="pse", bufs=1, space="PSUM") as pse:
                # ha^T = relu(Wa_aug^T @ aT_aug): [64, 256]
                p_haT = pse.tile([AH, 2 * R], f32, tag="phaT")
                nc.tensor.matmul(p_haT[:], WaA_t, aTA_t, start=True, stop=True)
                haT = wp.tile([AH, 2 * R], bf16, tag="haT")
                nc.scalar.activation(haT[:], p_haT[:], AF.Relu)

                # G planes -> g8 fp8 [128, 2*ZH]: plane0 = G1, plane1 = G0
                g8 = wp.tile([R, 2 * ZH], f8e4, tag="g8")
                p_g = pse.tile([R, ZH], f32, tag="pg")
                nc.tensor.matmul(p_g[:], haT[:, R:2 * R], W1a_t,
                                 start=True, stop=True)
                nc.scalar.copy(g8[:, 0:ZH], p_g[:])
                p_g2 = pse.tile([R, ZH], f32, tag="pg2")
                nc.tensor.matmul(p_g2[:], haT[:, 0:R], W1a_t,
                                 start=True, stop=True)
                nc.scalar.copy(g8[:, ZH:2 * ZH], p_g2[:])

                # u planes -> u8 fp8 [64, 2*ZH]: plane pl = u rows 64pl..
                u8 = wp.tile([64, 2 * ZH], f8e4, tag="u8")
                for pl in range(2):
                    p_u = pse.tile([64, ZH], f32, tag=f"pu{pl}")
                    for k in range(4):
                        nc.tensor.matmul(p_u[:], zzz_t[:, k, 64 * pl:64 * pl + 64],
                                         W1z_t[:, k, :],
                                         start=(k == 0), stop=False)
                    nc.tensor.matmul(p_u[:], ones_t[0:1, 0:64], b1_t,
                                     start=False, stop=True)
                    nc.scalar.copy(u8[:, ZH * pl:ZH * pl + ZH], p_u[:])

                # v = z_next @ W2^T -> bf16 (duplicated for batched TT)
                p_v = pse.tile([R, ZH], f32, tag="pv")
                for k in range(4):
                    nc.tensor.matmul(p_v[:], zzn_t[:, k, :], W2T_t[:, k, :],
                                     start=(k == 0), stop=(k == 3))
                v3 = wp.tile([R, SH_PER * ZH], bf16, tag="v3")
                nc.scalar.copy(v3[:, 0:ZH], p_v[:])
                nc.vector.tensor_copy(v3[:, ZH:2 * ZH], v3[:, 0:ZH])
                v_bf = v3[:, 0:ZH]

            # ---------------- neg-sim loop ----------------
            negsim = smp.tile([R, NSH], f32, tag="negsim")
            sim_ps = ps_sim.tile([R, B], f32, tag="simps")
            m1 = smp.tile([R, 1], f32, tag="m1")
            diag = smp.tile([R, 1], f32, tag="diag")
            Ssim = smp.tile([R, 1], f32, tag="Ssim")
            csim = smp.tile([R, 1], f32, tag="csim")
            nm1 = smp.tile([R, 1], f32, tag="nm1")
            z_t = zp_t[:, 0:Z]
            zn_t = zp_t[:, Z:2 * Z]

            pst_ctx = tc.tile_pool(name="pst", bufs=3, space="PSUM")
            pst = pst_ctx.__enter__()
            g8_3d = g8[:].rearrange("p (two z) -> p two z", two=2)
            u8_3d = u8[:].rearrange("p (two z) -> p two z", two=2)
            selu_3d = sel_t[0:64, 356:612].rearrange(
                "p (two m) -> p two m", two=2)

            def dr_shift(out_ap, s):
                lhsG = sel_t[:, s:s + 256].rearrange(
                    "p (two m) -> p two m", two=2)
                nc.tensor.matmul(out_ap, lhsG, g8_3d, start=True, stop=False,
                                 perf_mode=mybir.MatmulPerfMode.DoubleRow)
                nc.tensor.matmul(out_ap, selu_3d, u8_3d, start=False,
                                 stop=True,
                                 perf_mode=mybir.MatmulPerfMode.DoubleRow)

            # software pipeline: produce at step j, accumulate DEFER[route]
            # steps later so the in-order DVE queue never waits on a slow
            # producer (Pool mult 3.1us, X-route DMA 2.2us).
            DEFER = {"A": 2, "X": 1, "Z": 5}
            jobs = {}
            accum_at = {}
            for j in range(NMAC):
                accum_at.setdefault(j + DEFER[ROUTE_PATTERN[j]], []).append(j)
            max_step = NMAC + max(DEFER.values())

            def emit_sim_inserts(j):
                if j == 8:
                    # c0 = rowdot(z, z_next); base = c0 + c1
                    c0 = smp.tile([R, 1], f32, tag="c0")
                    c0scr = scrp.tile([R, Z], f32, tag="scr32")
                    nc.vector.scalar_tensor_tensor(
                        out=c0scr[:], in0=z_t, scalar=0.0, in1=zn_t,
                        op0=AL.add, op1=AL.mult, accum_out=c0[:])
                    base_t[0] = c0
                if j == 12:
                    for h in range(2):
                        for k in range(4):
                            nc.tensor.matmul(
                                sim_ps[:, h * 512:(h + 1) * 512],
                                zzn_t[:, k, :],
                                znhT_t[:, k, h * 512:(h + 1) * 512],
                                start=(k == 0), stop=(k == 3))
                if j == 17:
                    nc.vector.tensor_reduce(
                        m1[:], sim_ps[:], axis=mybir.AxisListType.X,
                        op=AL.max)
                    nc.vector.tensor_scalar_mul(nm1[:], m1[:], -INV_T)
                if j == 20:
                    eo = scrp.tile([R, B], f32, tag="scrB")
                    nc.scalar.activation(eo[:], sim_ps[:], AF.Exp,
                                         bias=nm1[:], scale=INV_T,
                                         accum_out=Ssim[:])
                    sim_sb = wp.tile([R, B], bf16, tag="simsb")
                    nc.scalar.copy(sim_sb[:], sim_ps[:])
                    sim_sb_t[0] = sim_sb
                if j == 22:
                    # diag via DRAM round-trip of the bf16 sim copy: dump,
                    # then gather the (b, b) diagonal as a stride-(DW+1)
                    # flat view (znhT host-rotated per core => diag at col b)
                    nc.sync.dma_start(simdump[0:R, 0:B], sim_sb_t[0][:])
                    flat = simdump[:, :].flatten()
                    dview = flat[0:R * (DW + 1)].rearrange(
                        "(p x) -> p x", x=DW + 1)
                    diag_bf = smp.tile([R, 1], bf16, tag="diagbf")
                    nc.sync.dma_start(diag_bf[:], dview[:, 0:1])
                    nc.vector.tensor_copy(diag[:], diag_bf[:])
                if j == 25:
                    co = scrp.tile([R, B], bf16, tag="scrBc")
                    nc.vector.tensor_scalar(
                        out=co[:], in0=sim_sb_t[0][:], scalar1=diag[:],
                        scalar2=0.0,
                        op0=AL.is_gt, op1=AL.add, accum_out=csim[:])

            base_t = [None]
            sim_sb_t = [None]
            for step in range(max_step + 1):
                if step < NMAC:
                    j = step
                    route = ROUTE_PATTERN[j]
                    nsh_j = min(SH_PER, NSH - SH_PER * j)
                    W = ZH * nsh_j
                    t3_ps = pst.tile([R, SH_PER * ZH], f32, tag="t3ps")
                    for k in range(nsh_j):
                        dr_shift(t3_ps[:, ZH * k:ZH * k + ZH],
                                 SH_PER * j + 1 + k)
                    if route == "X":
                        jobs[j] = ("X", t3_ps)
                    else:
                        t3 = t3p.tile([R, SH_PER * ZH], bf16, tag="t3")
                        nc.scalar.activation(t3[:, 0:W], t3_ps[:, 0:W],
                                             AF.Relu)
                        if route == "A":
                            jobs[j] = ("A", t3)
                        else:
                            rv3 = rv3p.tile([R, SH_PER * ZH], bf16,
                                            tag="rv3")
                            nc.gpsimd.tensor_mul(rv3[:, 0:W], t3[:, 0:W],
                                                 v3[:, 0:W])
                            jobs[j] = (route, rv3)
                for j2 in accum_at.get(step, ()):
                    route, buf = jobs.pop(j2)
                    if route == "A":
                        W2 = ZH * min(SH_PER, NSH - SH_PER * j2)
                        rv3 = rv3p.tile([R, SH_PER * ZH], bf16, tag="rv3")
                        nc.vector.tensor_tensor(rv3[:, 0:W2], buf[:, 0:W2],
                                                v3[:, 0:W2], op=AL.mult)
                        buf = rv3
                    for k in range(min(SH_PER, NSH - SH_PER * j2)):
                        col = negsim[:, SH_PER * j2 + k:SH_PER * j2 + k + 1]
                        if route == "X":
                            scr = scrp.tile([R, ZH], bf16, tag="scrx")
                            nc.vector.scalar_tensor_tensor(
                                out=scr[:], in0=buf[:, ZH * k:ZH * k + ZH],
                                scalar=0.0, in1=v_bf,
                                op0=AL.max, op1=AL.mult, accum_out=col)
                        else:
                            scr = scrp.tile([R, ZH], bf16, tag="scra")
                            nc.vector.tensor_scalar(
                                out=scr[:], in0=buf[:, ZH * k:ZH * k + ZH],
                                scalar1=1.0, scalar2=0.0,
                                op0=AL.mult, op1=AL.add, accum_out=col)
                if step < NMAC:
                    emit_sim_inserts(step)
            base = base_t[0]

            pst_ctx.__exit__(None, None, None)
            # ---------------- tail (neg side + combine) ----------------
            negf = smp.tile([R, NSH], f32, tag="negf")
            nc.vector.tensor_scalar_add(negf[:], negsim[:], base[:])

            m2 = smp.tile([R, 1], f32, tag="m2")
            nc.vector.tensor_reduce(m2[:], negf[:], axis=mybir.AxisListType.X,
                                    op=AL.max)
            m = smp.tile([R, 1], f32, tag="m")
            nc.vector.tensor_max(m[:], m1[:], m2[:])
            negb = smp.tile([R, 1], f32, tag="negb")
            nc.vector.tensor_scalar_mul(negb[:], m[:], -INV_T)

            eo = scrp.tile([R, NSH], f32, tag="scrn")
            Sneg = smp.tile([R, 1], f32, tag="Sneg")
            nc.scalar.activation(eo[:], negf[:], AF.Exp, bias=negb[:],
                                 scale=INV_T, accum_out=Sneg[:])
            dm = smp.tile([R, 1], f32, tag="dm")
            nc.vector.tensor_sub(dm[:], m1[:], m[:])
            r1 = smp.tile([R, 1], f32, tag="r1")
            nc.scalar.activation(r1[:], dm[:], AF.Exp, scale=INV_T)
            Ssc = smp.tile([R, 1], f32, tag="Ssc")
            nc.vector.tensor_mul(Ssc[:], Ssim[:], r1[:])
            S = smp.tile([R, 1], f32, tag="S")
            nc.vector.tensor_add(S[:], Ssc[:], Sneg[:])

            co = scrp.tile([R, NSH], f32, tag="scrn")
            cneg = smp.tile([R, 1], f32, tag="cneg")
            nc.vector.tensor_scalar(out=co[:], in0=negf[:],
                                    scalar1=diag[:], scalar2=0.0,
                                    op0=AL.is_gt, op1=AL.add,
                                    accum_out=cneg[:])
            cnt = smp.tile([R, 1], f32, tag="cnt")
            nc.vector.tensor_add(cnt[:], csim[:], cneg[:])

            out4 = smp.tile([R, 4], f32, tag="out4")
            nc.scalar.copy(out4[:, 0:1], m[:])
            nc.scalar.copy(out4[:, 1:2], S[:])
            nc.scalar.copy(out4[:, 2:3], diag[:])
            nc.scalar.copy(out4[:, 3:4], cnt[:])
            nc.sync.dma_start(d_out[:, :], out4[:])

    nc.compile()
    return nc


def _prepare_in_maps(z, z_next, z_next_hat, actions, Wa, ba, W1, b1, W2, b2):
    f = np.float32
    bf = ml_dtypes.bfloat16
    f8 = ml_dtypes.float8_e4m3fn
    z = np.ascontiguousarray(z, f)
    z_next = np.ascontiguousarray(z_next, f)
    z_next_hat = np.ascontiguousarray(z_next_hat, f)
    actions = np.ascontiguousarray(actions, f)

    def ktile(x):
        k = x.shape[0] // R
        return np.ascontiguousarray(
            x.reshape(k, R, -1).transpose(1, 0, 2).reshape(R, -1))

    znhT_mat = np.ascontiguousarray(z_next_hat.T)
    W2T_tiled = ktile(np.ascontiguousarray(W2.T)).astype(f8)
    W1z_tiled = ktile(np.ascontiguousarray(W1[:Z])).astype(f8)
    W1a = np.ascontiguousarray(W1[Z:]).astype(bf)
    Wa_aug = np.vstack([Wa, ba[None, :]]).astype(bf)
    W1a_pack = np.zeros((AH, ZH + 8), np.float32)
    W1a_pack[:, 0:ZH] = W1a.astype(np.float32)
    b1_row = b1.reshape(1, ZH).astype(bf)

    # fp8 selectors
    master = np.zeros((R, 356), f)
    master[np.arange(R), np.arange(R) + 128] = 1.0
    selu = np.zeros((64, 256), f)
    selu[np.arange(64), np.arange(64)] = 1.0
    selu[np.arange(64), 128 + 64 + np.arange(64)] = 1.0
    sel = np.zeros((R, 356 + 256), f)
    sel[:, 0:356] = master
    sel[0:64, 356:612] = selu
    sel = sel.astype(f8)

    in_maps = []
    for c in range(NCORES):
        s = c * R
        idx = (s + np.arange(2 * R)) % B
        a_sl = actions[idx]
        aT_aug = np.ascontiguousarray(
            np.vstack([a_sl.T, np.ones((1, 2 * R), f)])).astype(bf)
        dpos0 = np.arange(R, dtype=f).reshape(R, 1)
        znhT_tiled = ktile(np.ascontiguousarray(
            np.roll(znhT_mat, -s, axis=1))).astype(bf)
        zz_tiled = np.concatenate(
            [ktile(np.ascontiguousarray(z[s:s + R].T)),
             ktile(np.ascontiguousarray(z_next[s:s + R].T))],
            axis=1).astype(bf)
        small = np.zeros((A + 1, AH + 2 * R), bf)
        small[:, 0:AH] = Wa_aug
        small[:, AH:] = aT_aug
        zpack = np.concatenate(
            [z[s:s + R] + b2[None, :], z_next[s:s + R]], axis=1)
        in_maps.append({
            "small": small,
            "W1a_t": W1a_pack.astype(bf),
            "b1row": b1_row,
            "sel": sel,
            "zzn_tiled": zz_tiled[:, 4 * R:],
            "zzz_tiled": zz_tiled[:, 0:4 * R],
            "W1z_tiled": W1z_tiled,
            "W2T_tiled": W2T_tiled,
            "znhT_tiled": znhT_tiled,
            "zpack": np.ascontiguousarray(zpack, f),
        })
    return in_maps


def _finalize(results):
    m = np.concatenate([r["out4"][:, 0] for r in results]).astype(np.float64)
    S = np.concatenate([r["out4"][:, 1] for r in results]).astype(np.float64)
    diag = np.concatenate([r["out4"][:, 2] for r in results]).astype(np.float64)
    cnt = np.concatenate([r["out4"][:, 3] for r in results]).astype(np.float64)

    lse = m * INV_T + np.log(S)
    loss = np.float32(np.mean(lse - diag * INV_T))
    accs = [np.float32(np.mean(cnt < k)) for k in TOP_K]
    return (loss, accs[0], accs[1], accs[2])


def kernel(z, z_next, z_next_hat, actions, Wa, ba, W1, b1, W2, b2,
           _trace=False, _trace_kwargs=None):
    if "nc" not in _cache:
        _cache["nc"] = _build()
    nc = _cache["nc"]
    in_maps = _prepare_in_maps(z, z_next, z_next_hat, actions,
                               Wa, ba, W1, b1, W2, b2)
    kw = {}
    if _trace:
        kw = dict(trace=True, **(_trace_kwargs or {}))
    res = run_bass_kernel_spmd(nc, in_maps, core_ids=list(range(NCORES)), **kw)
    _cache["last_results"] = res.results
    out = _finalize(res.results)
    if _trace:
        return out, res
    return out


if __name__ == "__main__":
    rng = np.random.RandomState(0)
    args = dict(
        z=rng.randn(B, Z).astype(np.float32),
        z_next=rng.randn(B, Z).astype(np.float32),
        z_next_hat=rng.randn(B, Z).astype(np.float32),
        actions=rng.randn(B, A).astype(np.float32),
        Wa=(rng.randn(A, AH) / np.sqrt(A)).astype(np.float32),
        ba=np.zeros(AH, np.float32),
        W1=(rng.randn(Z + AH, ZH) / np.sqrt(Z + AH)).astype(np.float32),
        b1=np.zeros(ZH, np.float32),
        W2=(rng.randn(ZH, Z) / np.sqrt(ZH)).astype(np.float32),
        b2=np.zeros(Z, np.float32),
    )
    print(kernel(**args))


# revision 33
# speedup vs baseline: 1.0265x; 1.0265x over previous
"""ControlCPC loss kernel for 8 Trainium2 NeuronCores (Bass/Tile) — v2.

Row-sharded over the batch: core c owns rows [128c, 128c+128).

Algebraic reduction (B=1024, Z=512, A=8, AH=64, ZH=512, n_neg=100, T=0.1):
  sim[b, j] = z_next[b] . z_next_hat[j]
  u[b]      = z[b] @ W1[:Z] + b1
  G[j]      = relu(actions[j] @ Wa + ba) @ W1[Z:]      (j = s..s+255 local)
  v[b]      = z_next[b] @ W2.T
  neg[b, i] = z[b].z_next[b] + b2.z_next[b]
              + sum_zh relu(u[b,zh] + G[b+i,zh]) * v[b,zh]     i = 1..99
  row stats over x = [sim | neg]: m = rowmax, S = sum exp((x-m)/T),
  diag = sim[b,b] (exact from PSUM), cnt = #{x > diag}.
Host: loss = mean(m/T + ln S - diag/T), acc_k = mean(cnt < k).

v2 structure (vs the 79.6us baseline, cost-model-guided):
- t_i = u + G-window built by TWO fp8 DoubleRow matmuls per shift (~214ns
  hot vs 654 for 3 bf16 matmuls): rhs operands are plane-packed [p, 2, 512]
  (G1|G0 and u-halves), lhsT selectors are 3D slices of one diagonal
  master [128, 356] fp8 (master[p, p+128]=1), offset = shift.
- the relu.v reduce is spread over three engine routes in 3-shift macros:
    A: Act relu-copies the 3-bank PSUM region to bf16 SBUF (1465ns);
       DVE TT-mult by v (2x mode, 860) + 3 TS-accums (4x mode, 194).
    Z: Act relu-copy; Pool TT-mult (3143); 3 DVE TS-accums.
    X: 3 DVE fused STTs straight from PSUM (658 each).
  (Pool cannot touch PSUM or run STT/TS - BIR engine checks.)
- sim matmuls + stats and all input DMAs are overlapped into the loop.
fp8 e4m3 on u/G adds ~0.4 rms error per neg value; the loss tolerance
(rel 2e-2 on |loss|~887) and top-k margins absorb it (measured ~1e-3).
"""

import sys

for _p in ("/opt/trn_rl_repo", "/opt/pypackages"):
    if _p not in sys.path:
        sys.path.insert(0, _p)

import numpy as np
import ml_dtypes

import concourse.bass as bass
import concourse.mybir as mybir
import concourse.tile as tile
from concourse import bacc
from concourse.bass_utils import run_bass_kernel_spmd

f32 = mybir.dt.float32
bf16 = mybir.dt.bfloat16
f8e4 = mybir.dt.float8e4
AL = mybir.AluOpType
AF = mybir.ActivationFunctionType

B, Z, A = 1024, 512, 8
AH, ZH = 64, 512
TEMP = 0.1
NCORES = 8
R = B // NCORES          # 128 rows per core
NSH = 99                 # shifts 1..99
SH_PER = 2               # shifts per macro
NMAC = 50                # ceil(99/2); last macro carries 1 shift
INV_T = 1.0 / TEMP
TOP_K = (1, 3, 10)
DW = 1920                # simdump row pitch; diag at flat[b*(DW+1)]
SOFF = 0                 # znhT is host-rotated per core so diag sits at col b

# macro routes: A = Act+DVE chain, Z = Act+Pool chain, X = DVE fused-PSUM.
# Proportional interleave tuned against the TimelineSim cost model.
def _make_pattern(counts):
    err = {r: 0.0 for r in counts}
    total = sum(counts.values())
    out = []
    for _ in range(total):
        for r in counts:
            err[r] += counts[r] / total
        pick = max(err, key=lambda r: err[r])
        err[pick] -= 1.0
        out.append(pick)
    return "".join(out)

ROUTE_PATTERN = ("ZZ" + _make_pattern({"Z": 17, "A": 15, "X": 12}) + "AXAX")
assert len(ROUTE_PATTERN) == NMAC

_cache = {}


def _register_diag_op():
    from concourse.dve_ops import DveOp, OPS
    from concourse.dve_spec import Spec, Src0, C0, Zero, Idx, eq, select
    from operator import add

    for op in OPS:
        if op.name == "DIAG_EXTRACT_ANT":
            return op

    def _ref(in0, in1, s0, s1, imm2):
        in2 = in0.reshape(in0.shape[0], -1).astype(np.float32)
        idx = np.arange(in2.shape[1])[None, :]
        sel = np.where(idx == s0, in2, 0.0).astype(np.float32)
        return sel.reshape(in0.shape), sel.sum(axis=-1, keepdims=True)

    import concourse.dve_ops as dve_ops_mod
    from concourse.dve_spec import lower, _has_src1
    from concourse.dve_uop import DveOpSpec

    name = "DIAG_EXTRACT_ANT"
    spec = Spec(body=select(eq(Idx, C0), Src0, Zero), accum=add,
                accum_init=Zero, reference=_ref)
    row = max(dve_ops_mod._SUB_OPCODE_FOR_NAME.values()) + 1
    assert row < 0x20
    dve_ops_mod._SUB_OPCODE_FOR_NAME[name] = row

    shas = {}
    for ver in ("v3", "v4"):
        try:
            tmp = DveOpSpec(name=name, opcode=row, uops=lower(spec, ver=ver),
                            rd1_en=_has_src1(spec))
            shas[ver] = tmp.sha(ver)
        except Exception:
            pass
    op = DveOp(name, spec, subdim=False, uops_sha=shas)
    OPS.append(op)
    dve_ops_mod.CUSTOM_DVE_SPECS[name] = spec
    return op


def _build():
    diag_op = _register_diag_op()
    nc = bacc.Bacc("TRN2", target_bir_lowering=False, debug=False,
                   num_devices=NCORES)

    def din(name, shape, dtype=f32):
        return nc.dram_tensor(name, shape, dtype, kind="ExternalInput")

    d_small = din("small", [A + 1, AH + 2 * R], bf16)  # Wa_aug | aTA
    d_W1a = din("W1a_t", [AH, ZH + 8], bf16)           # W1a | b1 spread rows
    d_b1 = din("b1row", [1, ZH], bf16)
    d_sel = din("sel", [R, 356 + 256], f8e4)   # master | sel_u
    d_zzn = din("zzn_tiled", [R, 4 * R], bf16)  # z_next^T k-tiles
    d_zzz = din("zzz_tiled", [R, 4 * R], bf16)  # z^T k-tiles
    d_W1z = din("W1z_tiled", [R, 4 * ZH], f8e4)
    d_W2T = din("W2T_tiled", [R, 4 * ZH], f8e4)
    d_znhT = din("znhT_tiled", [R, 4 * B], bf16)
    d_zp = din("zpack", [R, 2 * Z])            # z+b2 | z_next

    d_out = nc.dram_tensor("out4", [R, 4], f32, kind="ExternalOutput")

    with tile.TileContext(nc) as tc:
        with tc.tile_pool(name="w", bufs=1) as wp, \
             tc.tile_pool(name="t3p", bufs=8) as t3p, \
             tc.tile_pool(name="rv3p", bufs=9) as rv3p, \
             tc.tile_pool(name="gdram", bufs=1, space="DRAM") as gdp, \
             tc.tile_pool(name="scr", bufs=6) as scrp, \
             tc.tile_pool(name="sm", bufs=1) as smp, \
             tc.tile_pool(name="pssim", bufs=1, space="PSUM") as ps_sim:

            # ---------------- input loads ----------------
            small_t = wp.tile([A + 1, AH + 2 * R], bf16, tag="small")
            nc.sync.dma_start(small_t[:], d_small[:, :])
            WaA_t = small_t[:, 0:AH]
            aTA_t = small_t[:, AH:AH + 2 * R]
            W1a_tile = wp.tile([AH, ZH + 8], bf16, tag="W1a")
            nc.sync.dma_start(W1a_tile[:], d_W1a[:, :])
            W1a_t = W1a_tile[:, 0:ZH]
            b1_tile = wp.tile([1, ZH], bf16, tag="b1")
            nc.sync.dma_start(b1_tile[:], d_b1[:, :])
            b1_t = b1_tile[0:1, :]
            W1z_t = wp.tile([R, 4, ZH], f8e4, tag="W1z")
            nc.sync.dma_start(W1z_t[:], d_W1z[:, :].rearrange(
                "p (k m) -> p k m", k=4))
            zzz_t = wp.tile([R, 4, R], bf16, tag="zzz")
            nc.sync.dma_start(zzz_t[:], d_zzz[:, :].rearrange(
                "p (k m) -> p k m", k=4))
            sel_t = wp.tile([R, 356 + 256], f8e4, tag="sel")
            nc.sync.dma_start(sel_t[:], d_sel[:, :])
            W2T_t = wp.tile([R, 4, ZH], f8e4, tag="W2T")
            nc.sync.dma_start(W2T_t[:], d_W2T[:, :].rearrange(
                "p (k m) -> p k m", k=4))
            zzn_t = wp.tile([R, 4, R], bf16, tag="zzn")
            nc.sync.dma_start(zzn_t[:], d_zzn[:, :].rearrange(
                "p (k m) -> p k m", k=4))
            znhT_t = wp.tile([R, 4, B], bf16, tag="znhT")
            nc.sync.dma_start(znhT_t[:], d_znhT[:, :].rearrange(
                "p (k m) -> p k m", k=4))
            zp_t = wp.tile([R, 2 * Z], f32, tag="zp")
            nc.sync.dma_start(zp_t[:], d_zp[:, :])

            ones_t = wp.tile([1, R], bf16, tag="ones")
            nc.gpsimd.memset(ones_t[:], 1.0)

            # DRAM scratch for the sim diagonal gather (see emit j==22):
            # rows padded to DW so flat[b*(DW+1)] == dump[b, b].
            simdump = gdp.tile([R + 1, DW], bf16, tag="simdump")

            # ---------------- setup compute ----------------
            with tc.tile_pool(name="pse", bufs=1, space="PSUM") as pse:
                # ha^T = relu(Wa_aug^T @ aT_aug): [64, 256]
                p_haT = pse.tile([AH, 2 * R], f32, tag="phaT")
                nc.tensor.matmul(p_haT[:], WaA_t, aTA_t, start=True, stop=True)
                haT = wp.tile([AH, 2 * R], bf16, tag="haT")
                nc.scalar.activation(haT[:], p_haT[:], AF.Relu)

                # G planes -> g8 fp8 [128, 2*ZH]: plane0 = G1, plane1 = G0
                g8 = wp.tile([R, 2 * ZH], f8e4, tag="g8")
                p_g = pse.tile([R, ZH], f32, tag="pg")
                nc.tensor.matmul(p_g[:], haT[:, R:2 * R], W1a_t,
                                 start=True, stop=True)
                nc.scalar.copy(g8[:, 0:ZH], p_g[:])
                p_g2 = pse.tile([R, ZH], f32, tag="pg2")
                nc.tensor.matmul(p_g2[:], haT[:, 0:R], W1a_t,
                                 start=True, stop=True)
                nc.scalar.copy(g8[:, ZH:2 * ZH], p_g2[:])

                # u planes -> u8 fp8 [64, 2*ZH]: plane pl = u rows 64pl..
                u8 = wp.tile([64, 2 * ZH], f8e4, tag="u8")
                for pl in range(2):
                    p_u = pse.tile([64, ZH], f32, tag=f"pu{pl}")
                    for k in range(4):
                        nc.tensor.matmul(p_u[:], zzz_t[:, k, 64 * pl:64 * pl + 64],
                                         W1z_t[:, k, :],
                                         start=(k == 0), stop=False)
                    nc.tensor.matmul(p_u[:], ones_t[0:1, 0:64], b1_t,
                                     start=False, stop=True)
                    nc.scalar.copy(u8[:, ZH * pl:ZH * pl + ZH], p_u[:])

                # v = z_next @ W2^T -> bf16 (duplicated for batched TT)
                p_v = pse.tile([R, ZH], f32, tag="pv")
                for k in range(4):
                    nc.tensor.matmul(p_v[:], zzn_t[:, k, :], W2T_t[:, k, :],
                                     start=(k == 0), stop=(k == 3))
                v3 = wp.tile([R, SH_PER * ZH], bf16, tag="v3")
                nc.scalar.copy(v3[:, 0:ZH], p_v[:])
                nc.vector.tensor_copy(v3[:, ZH:2 * ZH], v3[:, 0:ZH])
                v_bf = v3[:, 0:ZH]

            # ---------------- neg-sim loop ----------------
            negsim = smp.tile([R, NSH], f32, tag="negsim")
            sim_ps = ps_sim.tile([R, B], f32, tag="simps")
            m1 = smp.tile([R, 1], f32, tag="m1")
            diag = smp.tile([R, 1], f32, tag="diag")
            Ssim = smp.tile([R, 1], f32, tag="Ssim")
            csim = smp.tile([R, 1], f32, tag="csim")
            nm1 = smp.tile([R, 1], f32, tag="nm1")
            z_t = zp_t[:, 0:Z]
            zn_t = zp_t[:, Z:2 * Z]

            pst_ctx = tc.tile_pool(name="pst", bufs=3, space="PSUM")
            pst = pst_ctx.__enter__()
            g8_3d = g8[:].rearrange("p (two z) -> p two z", two=2)
            u8_3d = u8[:].rearrange("p (two z) -> p two z", two=2)
            selu_3d = sel_t[0:64, 356:612].rearrange(
                "p (two m) -> p two m", two=2)

            def dr_shift(out_ap, s):
                lhsG = sel_t[:, s:s + 256].rearrange(
                    "p (two m) -> p two m", two=2)
                nc.tensor.matmul(out_ap, lhsG, g8_3d, start=True, stop=False,
                                 perf_mode=mybir.MatmulPerfMode.DoubleRow)
                nc.tensor.matmul(out_ap, selu_3d, u8_3d, start=False,
                                 stop=True,
                                 perf_mode=mybir.MatmulPerfMode.DoubleRow)

            # software pipeline: produce at step j, accumulate DEFER[route]
            # steps later so the in-order DVE queue never waits on a slow
            # producer (Pool mult 3.1us, X-route DMA 2.2us).
            DEFER = {"A": 2, "X": 1, "Z": 5}
            jobs = {}
            accum_at = {}
            for j in range(NMAC):
                accum_at.setdefault(j + DEFER[ROUTE_PATTERN[j]], []).append(j)
            max_step = NMAC + max(DEFER.values())

            def emit_sim_inserts(j):
                if j == 8:
                    # c0 = rowdot(z, z_next); base = c0 + c1
                    c0 = smp.tile([R, 1], f32, tag="c0")
                    c0scr = scrp.tile([R, Z], f32, tag="scr32")
                    nc.vector.scalar_tensor_tensor(
                        out=c0scr[:], in0=z_t, scalar=0.0, in1=zn_t,
                        op0=AL.add, op1=AL.mult, accum_out=c0[:])
                    base_t[0] = c0
                if j == 12:
                    for h in range(2):
                        for k in range(4):
                            nc.tensor.matmul(
                                sim_ps[:, h * 512:(h + 1) * 512],
                                zzn_t[:, k, :],
                                znhT_t[:, k, h * 512:(h + 1) * 512],
                                start=(k == 0), stop=(k == 3))
                if j == 17:
                    nc.vector.tensor_reduce(
                        m1[:], sim_ps[:], axis=mybir.AxisListType.X,
                        op=AL.max)
                    nc.vector.tensor_scalar_mul(nm1[:], m1[:], -INV_T)
                if j == 20:
                    eo = scrp.tile([R, B], f32, tag="scrB")
                    nc.scalar.activation(eo[:], sim_ps[:], AF.Exp,
                                         bias=nm1[:], scale=INV_T,
                                         accum_out=Ssim[:])
                    sim_sb = wp.tile([R, B], bf16, tag="simsb")
                    nc.scalar.copy(sim_sb[:], sim_ps[:])
                    sim_sb_t[0] = sim_sb
                if j == 22:
                    # diag via DRAM round-trip of the bf16 sim copy: dump,
                    # then gather the (b, b) diagonal as a stride-(DW+1)
                    # flat view (znhT host-rotated per core => diag at col b)
                    nc.sync.dma_start(simdump[0:R, 0:B], sim_sb_t[0][:])
                    flat = simdump[:, :].flatten()
                    dview = flat[0:R * (DW + 1)].rearrange(
                        "(p x) -> p x", x=DW + 1)
                    diag_bf = smp.tile([R, 1], bf16, tag="diagbf")
                    nc.sync.dma_start(diag_bf[:], dview[:, 0:1])
                    nc.vector.tensor_copy(diag[:], diag_bf[:])
                if j == 25:
                    co = scrp.tile([R, B], bf16, tag="scrBc")
                    nc.vector.tensor_scalar(
                        out=co[:], in0=sim_sb_t[0][:], scalar1=diag[:],
                        scalar2=0.0,
                        op0=AL.is_gt, op1=AL.add, accum_out=csim[:])

            base_t = [None]
            sim_sb_t = [None]
            for step in range(max_step + 1):
                if step < NMAC:
                    j = step
                    route = ROUTE_PATTERN[j]
                    nsh_j = min(SH_PER, NSH - SH_PER * j)
                    W = ZH * nsh_j
                    t3_ps = pst.tile([R, SH_PER * ZH], f32, tag="t3ps")
                    for k in range(nsh_j):
                        dr_shift(t3_ps[:, ZH * k:ZH * k + ZH],
                                 SH_PER * j + 1 + k)
                    if route == "X":
                        jobs[j] = ("X", t3_ps)
                    else:
                        t3 = t3p.tile([R, SH_PER * ZH], bf16, tag="t3")
                        nc.scalar.activation(t3[:, 0:W], t3_ps[:, 0:W],
                                             AF.Relu)
                        if route == "A":
                            jobs[j] = ("A", t3)
                        else:
                            rv3 = rv3p.tile([R, SH_PER * ZH], bf16,
                                            tag="rv3")
                            nc.gpsimd.tensor_mul(rv3[:, 0:W], t3[:, 0:W],
                                                 v3[:, 0:W])
                            jobs[j] = (route, rv3)
                for j2 in accum_at.get(step, ()):
                    route, buf = jobs.pop(j2)
                    if route == "A":
                        W2 = ZH * min(SH_PER, NSH - SH_PER * j2)
                        rv3 = rv3p.tile([R, SH_PER * ZH], bf16, tag="rv3")
                        nc.vector.tensor_tensor(rv3[:, 0:W2], buf[:, 0:W2],
                                                v3[:, 0:W2], op=AL.mult)
                        buf = rv3
                    for k in range(min(SH_PER, NSH - SH_PER * j2)):
                        col = negsim[:, SH_PER * j2 + k:SH_PER * j2 + k + 1]
                        if route == "X":
                            scr = scrp.tile([R, ZH], bf16, tag="scrx")
                            nc.vector.scalar_tensor_tensor(
                                out=scr[:], in0=buf[:, ZH * k:ZH * k + ZH],
                                scalar=0.0, in1=v_bf,
                                op0=AL.max, op1=AL.mult, accum_out=col)
                        else:
                            scr = scrp.tile([R, ZH], bf16, tag="scra")
                            nc.vector.tensor_scalar(
                                out=scr[:], in0=buf[:, ZH * k:ZH * k + ZH],
                                scalar1=1.0, scalar2=0.0,
                                op0=AL.mult, op1=AL.add, accum_out=col)
                if step < NMAC:
                    emit_sim_inserts(step)
            base = base_t[0]

            pst_ctx.__exit__(None, None, None)
            # ---------------- tail (neg side + combine) ----------------
            negf = smp.tile([R, NSH], f32, tag="negf")
            nc.vector.tensor_scalar_add(negf[:], negsim[:], base[:])

            m2 = smp.tile([R, 1], f32, tag="m2")
            nc.vector.tensor_reduce(m2[:], negf[:], axis=mybir.AxisListType.X,
                                    op=AL.max)
            m = smp.tile([R, 1], f32, tag="m")
            nc.vector.tensor_max(m[:], m1[:], m2[:])
            negb = smp.tile([R, 1], f32, tag="negb")
            nc.vector.tensor_scalar_mul(negb[:], m[:], -INV_T)

            eo = scrp.tile([R, NSH], f32, tag="scrn")
            Sneg = smp.tile([R, 1], f32, tag="Sneg")
            nc.scalar.activation(eo[:], negf[:], AF.Exp, bias=negb[:],
                                 scale=INV_T, accum_out=Sneg[:])
            dm = smp.tile([R, 1], f32, tag="dm")
            nc.vector.tensor_sub(dm[:], m1[:], m[:])
            r1 = smp.tile([R, 1], f32, tag="r1")
            nc.scalar.activation(r1[:], dm[:], AF.Exp, scale=INV_T)
            Ssc = smp.tile([R, 1], f32, tag="Ssc")
            nc.vector.tensor_mul(Ssc[:], Ssim[:], r1[:])
            S = smp.tile([R, 1], f32, tag="S")
            nc.vector.tensor_add(S[:], Ssc[:], Sneg[:])

            co = scrp.tile([R, NSH], f32, tag="scrn")
            cneg = smp.tile([R, 1], f32, tag="cneg")
            nc.vector.tensor_scalar(out=co[:], in0=negf[:],
                                    scalar1=diag[:], scalar2=0.0,
                                    op0=AL.is_gt, op1=AL.add,
                                    accum_out=cneg[:])
            cnt = smp.tile([R, 1], f32, tag="cnt")
            nc.vector.tensor_add(cnt[:], csim[:], cneg[:])

            out4 = smp.tile([R, 4], f32, tag="out4")
            nc.scalar.copy(out4[:, 0:1], m[:])
            nc.scalar.copy(out4[:, 1:2], S[:])
            nc.scalar.copy(out4[:, 2:3], diag[:])
            nc.scalar.copy(out4[:, 3:4], cnt[:])
            nc.sync.dma_start(d_out[:, :], out4[:])

    nc.compile()
    return nc


def _prepare_in_maps(z, z_next, z_next_hat, actions, Wa, ba, W1, b1, W2, b2):
    f = np.float32
    bf = ml_dtypes.bfloat16
    f8 = ml_dtypes.float8_e4m3fn
    z = np.ascontiguousarray(z, f)
    z_next = np.ascontiguousarray(z_next, f)
    z_next_hat = np.ascontiguousarray(z_next_hat, f)
    actions = np.ascontiguousarray(actions, f)

    def ktile(x):
        k = x.shape[0] // R
        return np.ascontiguousarray(
            x.reshape(k, R, -1).transpose(1, 0, 2).reshape(R, -1))

    znhT_mat = np.ascontiguousarray(z_next_hat.T)
    W2T_tiled = ktile(np.ascontiguousarray(W2.T)).astype(f8)
    W1z_tiled = ktile(np.ascontiguousarray(W1[:Z])).astype(f8)
    W1a = np.ascontiguousarray(W1[Z:]).astype(bf)
    Wa_aug = np.vstack([Wa, ba[None, :]]).astype(bf)
    W1a_pack = np.zeros((AH, ZH + 8), np.float32)
    W1a_pack[:, 0:ZH] = W1a.astype(np.float32)
    b1_row = b1.reshape(1, ZH).astype(bf)

    # fp8 selectors
    master = np.zeros((R, 356), f)
    master[np.arange(R), np.arange(R) + 128] = 1.0
    selu = np.zeros((64, 256), f)
    selu[np.arange(64), np.arange(64)] = 1.0
    selu[np.arange(64), 128 + 64 + np.arange(64)] = 1.0
    sel = np.zeros((R, 356 + 256), f)
    sel[:, 0:356] = master
    sel[0:64, 356:612] = selu
    sel = sel.astype(f8)

    in_maps = []
    for c in range(NCORES):
        s = c * R
        idx = (s + np.arange(2 * R)) % B
        a_sl = actions[idx]
        aT_aug = np.ascontiguousarray(
            np.vstack([a_sl.T, np.ones((1, 2 * R), f)])).astype(bf)
        dpos0 = np.arange(R, dtype=f).reshape(R, 1)
        znhT_tiled = ktile(np.ascontiguousarray(
            np.roll(znhT_mat, -s, axis=1))).astype(bf)
        zz_tiled = np.concatenate(
            [ktile(np.ascontiguousarray(z[s:s + R].T)),
             ktile(np.ascontiguousarray(z_next[s:s + R].T))],
            axis=1).astype(bf)
        small = np.zeros((A + 1, AH + 2 * R), bf)
        small[:, 0:AH] = Wa_aug
        small[:, AH:] = aT_aug
        zpack = np.concatenate(
            [z[s:s + R] + b2[None, :], z_next[s:s + R]], axis=1)
        in_maps.append({
            "small": small,
            "W1a_t": W1a_pack.astype(bf),
            "b1row": b1_row,
            "sel": sel,
            "zzn_tiled": zz_tiled[:, 4 * R:],
            "zzz_tiled": zz_tiled[:, 0:4 * R],
            "W1z_tiled": W1z_tiled,
            "W2T_tiled": W2T_tiled,
            "znhT_tiled": znhT_tiled,
            "zpack": np.ascontiguousarray(zpack, f),
        })
    return in_maps


def _finalize(results):
    m = np.concatenate([r["out4"][:, 0] for r in results]).astype(np.float64)
    S = np.concatenate([r["out4"][:, 1] for r in results]).astype(np.float64)
    diag = np.concatenate([r["out4"][:, 2] for r in results]).astype(np.float64)
    cnt = np.concatenate([r["out4"][:, 3] for r in results]).astype(np.float64)

    lse = m * INV_T + np.log(S)
    loss = np.float32(np.mean(lse - diag * INV_T))
    accs = [np.float32(np.mean(cnt < k)) for k in TOP_K]
    return (loss, accs[0], accs[1], accs[2])


def kernel(z, z_next, z_next_hat, actions, Wa, ba, W1, b1, W2, b2,
           _trace=False, _trace_kwargs=None):
    if "nc" not in _cache:
        _cache["nc"] = _build()
    nc = _cache["nc"]
    in_maps = _prepare_in_maps(z, z_next, z_next_hat, actions,
                               Wa, ba, W1, b1, W2, b2)
    kw = {}
    if _trace:
        kw = dict(trace=True, **(_trace_kwargs or {}))
    res = run_bass_kernel_spmd(nc, in_maps, core_ids=list(range(NCORES)), **kw)
    _cache["last_results"] = res.results
    out = _finalize(res.results)
    if _trace:
        return out, res
    return out


if __name__ == "__main__":
    rng = np.random.RandomState(0)
    args = dict(
        z=rng.randn(B, Z).astype(np.float32),
        z_next=rng.randn(B, Z).astype(np.float32),
        z_next_hat=rng.randn(B, Z).astype(np.float32),
        actions=rng.randn(B, A).astype(np.float32),
        Wa=(rng.randn(A, AH) / np.sqrt(A)).astype(np.float32),
        ba=np.zeros(AH, np.float32),
        W1=(rng.randn(Z + AH, ZH) / np.sqrt(Z + AH)).astype(np.float32),
        b1=np.zeros(ZH, np.float32),
        W2=(rng.randn(ZH, Z) / np.sqrt(ZH)).astype(np.float32),
        b2=np.zeros(Z, np.float32),
    )
    print(kernel(**args))
